# revision 1
# baseline (speedup 1.0000x reference)
"""Distributed triangle multiplication (AlphaFold-style) for 8 Trainium2 NeuronCores.

Pipeline (per core, SPMD — one program, inputs sharded by the host):
  Phase 1 (data-parallel over pair rows, 96 rows/core):
    LN1 stats via bn_stats/bn_aggr (pos-major), fused normalize+cast to bf16
    (ln1_w/b folded into downstream weights), stage xn pos-major, xbar-transpose
    reload channel-major, PE matmul vs [proj|gate] folded weights, sigmoid gate,
    write 256 product channels (channel-major bf16).
  AllToAll #1 (x2, split by channel half): row-sharded -> channel-sharded.
  Phase 2 (tensor-parallel, 16 einsum channels/core):
    xbar-transpose products into [k, i] layout, batched 768^3 bf16 matmuls
    per channel (outgoing triangle multiply), write out2 row-block-sharded.
  AllToAll #2 (x2, split by position half): channel-sharded -> row-sharded.
  Phase 3 (data-parallel):
    xbar-transpose to pos-major, LN2 stats + normalize (ln2_w folded), stage y,
    xbar reload channel-major, PE matmul vs [out_w|gating_w] folded weights
    (pos-major output), sigmoid * product epilogue, store f32 pos-major.
"""
import sys

for _p in ("/opt/trn_rl_repo", "/opt/trn_rl_repo/concourse"):
    if _p not in sys.path:
        sys.path.insert(0, _p)

import os
import numpy as np
import ml_dtypes

import concourse.bass as bass
import concourse.tile as tile
from concourse import bacc, mybir
from concourse.bass_utils import run_bass_kernel_spmd

F32 = mybir.dt.float32
BF16 = mybir.dt.bfloat16
AF = mybir.ActivationFunctionType
ALU = mybir.AluOpType

N = 768
C = 128
NCORES = 8
R = N // NCORES            # 96 rows per core
POS = R * N                # 73728 positions per core
EPS = 1e-5
GS = 8                     # stats batching group

_PROGRAM_CACHE = {}
LAST_EXEC_NS = None
LAST_TRACE = None


def _build_program(flags, sim=False, upto=5):
    use_c1, use_cfin, use_mask = flags
    nc = bacc.Bacc("TRN2", target_bir_lowering=False, debug=False,
                   num_devices=1 if sim else NCORES)

    def _collective(ins, outs):
        if sim:
            nc.sync.dma_start(out=outs[0], in_=ins[0])
        else:
            nc.gpsimd.collective_compute("AllToAll", ALU.bypass,
                                         [list(range(NCORES))],
                                         ins=ins, outs=outs)

    pair_r = nc.dram_tensor("pair_r", [POS, C], F32, kind="ExternalInput").ap()
    w1t = nc.dram_tensor("w1t", [C, 4 * C], BF16, kind="ExternalInput").ap()
    wfin = nc.dram_tensor("wfin", [C, 2 * C], BF16, kind="ExternalInput").ap()
    if use_c1:
        c1r = nc.dram_tensor("c1r", [1, 4 * C], BF16, kind="ExternalInput").ap()
    if use_cfin:
        cfr = nc.dram_tensor("cfr", [1, 2 * C], BF16, kind="ExternalInput").ap()
    if use_mask:
        mask_r = nc.dram_tensor("mask_r", [POS // 512, 512], F32,
                                kind="ExternalInput").ap()

    xn1 = nc.dram_tensor("xn1", [POS, C], BF16).ap()
    # A2A#1 buffers, one per channel half. Rows: (shard s, 16 channels).
    abA_i = nc.dram_tensor("abA_i", [C, POS], BF16).ap()
    abB_i = nc.dram_tensor("abB_i", [C, POS], BF16).ap()
    abA_o = nc.dram_tensor("abA_o", [C, POS], BF16).ap()
    abB_o = nc.dram_tensor("abB_o", [C, POS], BF16).ap()
    # A2A#2 buffers. Rows: (shard s, 16 einsum channels).
    o2h_i = [nc.dram_tensor(f"o2h{h}_i", [C, POS // 2], BF16).ap()
             for h in range(2)]
    o2h_o = [nc.dram_tensor(f"o2h{h}_o", [C, POS // 2], BF16).ap()
             for h in range(2)]
    y_d = nc.dram_tensor("y_d", [POS, C], BF16).ap()
    out_r = nc.dram_tensor("out_r", [POS, C], F32, kind="ExternalOutput").ap()

    with tile.TileContext(nc) as tc:
        # ---- constants ----
        with tc.tile_pool(name="consts", bufs=1) as cpool:
            w1sb = cpool.tile([C, 4 * C], BF16)
            nc.sync.dma_start(w1sb[:], w1t[:, :])
            wfsb = cpool.tile([C, 2 * C], BF16)
            nc.sync.dma_start(wfsb[:], wfin[:, :])
            epsb = cpool.tile([128, 1], F32)
            nc.vector.memset(epsb[:], EPS)
            if use_c1 or use_cfin:
                ones_sb = cpool.tile([1, 512], BF16)
                nc.vector.memset(ones_sb[:], 1.0)
            if use_c1:
                c1sb = cpool.tile([1, 4 * C], BF16)
                nc.sync.dma_start(c1sb[:], c1r[:, :])
            if use_cfin:
                cfsb = cpool.tile([1, 2 * C], BF16)
                nc.sync.dma_start(cfsb[:], cfr[:, :])

            # ================= Phase 1 =================
            NT = POS // 128           # 576 pos-tiles
            with tc.tile_pool(name="p1x", bufs=2 * GS + 2) as p1x, \
                 tc.tile_pool(name="p1st", bufs=2 * GS + 4) as p1st, \
                 tc.tile_pool(name="p1mv", bufs=3) as p1mv, \
                 tc.tile_pool(name="p1xn", bufs=6) as p1xn:
                for g in range(NT // GS):
                    mv = p1mv.tile([128, 2 * GS], F32, tag="mv")
                    xts = []
                    for j in range(GS):
                        t = g * GS + j
                        xt = p1x.tile([128, C], F32, tag="xt")
                        nc.sync.dma_start(xt[:], pair_r[t * 128:(t + 1) * 128, :])
                        st6 = p1st.tile([128, 6], F32, tag="st6")
                        nc.vector.bn_stats(st6[:], xt[:])
                        nc.vector.bn_aggr(mv[:, 2 * j:2 * j + 2], st6[:])
                        xts.append(xt)
                    std = p1mv.tile([128, GS], F32, tag="std")
                    nc.scalar.activation(std[:], mv[:, 1:2 * GS:2], AF.Sqrt,
                                         bias=epsb[:])
                    rr = p1mv.tile([128, GS], F32, tag="rr")
                    nc.vector.reciprocal(rr[:], std[:])
                    nmr = p1mv.tile([128, GS], F32, tag="nmr")
                    nc.vector.tensor_mul(nmr[:], mv[:, 0:2 * GS:2], rr[:])
                    nc.vector.tensor_scalar_mul(nmr[:], nmr[:], -1.0)
                    for j in range(GS):
                        t = g * GS + j
                        xnt = p1xn.tile([128, C], BF16, tag="xn")
                        nc.vector.tensor_scalar(
                            xnt[:], xts[j][:], rr[:, j:j + 1], nmr[:, j:j + 1],
                            ALU.mult, ALU.add)
                        nc.sync.dma_start(xn1[t * 128:(t + 1) * 128, :], xnt[:])

            # projection matmuls over 512-pos chunks (two per xbar load)
            NCK = POS // 512          # 144 chunks
            with tc.tile_pool(name="p1xT", bufs=3) as p1xT, \
                 tc.tile_pool(name="p1sig", bufs=3) as p1sig, \
                 tc.tile_pool(name="p1ab", bufs=4) as p1ab, \
                 tc.tile_pool(name="p1ps", bufs=2, space="PSUM") as p1ps:
                for q in range(NCK // 2):
                    xT = p1xT.tile([C, 1024], BF16, tag="xT")
                    nc.sync.dma_start(xT[:], xn1[q * 1024:(q + 1) * 1024, :],
                                      transpose=True)
                    for h in range(2):
                        ck = 2 * q + h
                        ps = p1ps.tile([128, 4, 512], F32, tag="ps")
                        rhs = xT[:, h * 512:(h + 1) * 512]
                        for ob in range(4):
                            nc.tensor.matmul(
                                ps[:, ob, :], w1sb[:, ob * 128:(ob + 1) * 128],
                                rhs, start=True, stop=not use_c1)
                        if use_c1:
                            for ob in range(4):
                                nc.tensor.matmul(
                                    ps[:, ob, :],
                                    c1sb[:, ob * 128:(ob + 1) * 128],
                                    ones_sb[:, 0:512], start=False, stop=True)
                        sig = p1sig.tile([128, 2, 512], F32, tag="sig")
                        nc.scalar.activation(sig[:], ps[:, 2:4, :], AF.Sigmoid)
                        if use_mask:
                            mrow = p1sig.tile([1, 512], F32, tag="mrow")
                            nc.sync.dma_start(mrow[:], mask_r[ck:ck + 1, :])
                            mb = p1sig.tile([128, 512], F32, tag="mb")
                            nc.gpsimd.partition_broadcast(mb[:], mrow[:])
                            nc.vector.tensor_mul(
                                sig[:], sig[:],
                                mb[:].to_broadcast([128, 2, 512]))
                        abt = p1ab.tile([128, 2, 512], BF16, tag="abt")
                        nc.vector.tensor_mul(abt[:], ps[:, 0:2, :], sig[:])
                        # route channel o to (s=o//32, u=o%32); u<16 -> A else B
                        # rows of abt part0 = channels 0..127, part1 = 128..255
                        for part in range(2):
                            for s4 in range(4):
                                o0 = part * 128 + s4 * 32
                                sblk = o0 // 32
                                nc.sync.dma_start(
                                    abA_i[sblk * 16:sblk * 16 + 16,
                                          ck * 512:(ck + 1) * 512],
                                    abt[s4 * 32:s4 * 32 + 16, part, :])
                                nc.sync.dma_start(
                                    abB_i[sblk * 16:sblk * 16 + 16,
                                          ck * 512:(ck + 1) * 512],
                                    abt[s4 * 32 + 16:s4 * 32 + 32, part, :])

            # ================= A2A #1 =================
            if upto >= 2:
                _collective([abA_i[:]], [abA_o[:]])
                _collective([abB_i[:]], [abB_o[:]])

            # ================= Phase 2 =================
            # ab*_o rows: 16*r + u = product channel (32*me + H*16 + u) at core
            # r's rows. einsum channel c_local = (16*me+cl); a=2*cl', b=2*cl'+1
            # within each half's 16 rows (cl' = 0..7).
            KC = N // 128             # 6 k-chunks
            if upto >= 3:
             with tc.tile_pool(name="p2bt", bufs=2) as p2bt, \
                 tc.tile_pool(name="p2o", bufs=4) as p2o, \
                 tc.tile_pool(name="p2ps", bufs=2, space="PSUM") as p2ps:
                for half, ab_o in ((0, abA_o), (1, abB_o)):
                    abv = ab_o.rearrange("(r s u) (i k) -> r s (u i) k",
                                         r=NCORES, s=4, i=R)
                    for sub in range(4):      # 4 ch-rows (2 einsum ch) per sub
                        bt = p2bt.tile([128, KC, NCORES, 4, R], BF16, tag="bt")
                        for kc in range(KC):
                            for r in range(NCORES):
                                nc.sync.dma_start(
                                    bt[:, kc, r, :, :].rearrange(
                                        "p a b -> p (a b)"),
                                    abv[r, sub, :,
                                        kc * 128:(kc + 1) * 128],
                                    transpose=True)
                        for u in range(2):    # einsum channels in this sub
                            cl = half * 8 + sub * 2 + u
                            for it in range(NCORES):
                                ps = p2ps.tile([R, 2, 512], F32, tag="ps2")
                                for kc in range(KC):
                                    lhsT = bt[:, kc, it, 2 * u, :]
                                    rhsf = bt[:, kc, :, 2 * u + 1, :]
                                    nc.tensor.matmul(
                                        ps[:, 0, 0:384], lhsT,
                                        rhsf[:, 0:4, :],
                                        start=(kc == 0), stop=(kc == KC - 1))
                                    nc.tensor.matmul(
                                        ps[:, 1, 0:384], lhsT,
                                        rhsf[:, 4:8, :],
                                        start=(kc == 0), stop=(kc == KC - 1))
                                ot = p2o.tile([R, 2, 512], BF16, tag="ot")
                                if (it + u) % 2 == 0:
                                    nc.vector.tensor_copy(ot[:], ps[:])
                                else:
                                    nc.scalar.activation(ot[:], ps[:], AF.Copy)
                                # store halves for the split A2A#2
                                # rows 0:48 -> half 0, 48:96 -> half 1;
                                # j split across the two 384-col bank regions
                                for hh in range(2):
                                    dst = o2h_i[hh].rearrange(
                                        "(s c) (i j) -> s c i j",
                                        s=NCORES, i=R // 2)
                                    r0 = hh * (R // 2)
                                    nc.sync.dma_start(
                                        dst[it, cl, :, 0:384],
                                        ot[r0:r0 + R // 2, 0, 0:384])
                                    nc.sync.dma_start(
                                        dst[it, cl, :, 384:768],
                                        ot[r0:r0 + R // 2, 1, 0:384])

            # ================= A2A #2 =================
            if upto >= 4:
                for hh in range(2):
                    _collective([o2h_i[hh][:]], [o2h_o[hh][:]])

            # ================= Phase 3 =================
            if upto >= 4.5:
             with tc.tile_pool(name="p3t", bufs=2 * GS + 2) as p3t, \
                 tc.tile_pool(name="p3st", bufs=2 * GS + 4) as p3st, \
                 tc.tile_pool(name="p3mv", bufs=3) as p3mv, \
                 tc.tile_pool(name="p3y", bufs=6) as p3y:
                for g in range(NT // GS):
                    mv = p3mv.tile([128, 2 * GS], F32, tag="mv3")
                    o2ts = []
                    for j in range(GS):
                        t = g * GS + j
                        o2t = p3t.tile([128, C], BF16, tag="o2t")
                        hh, tt = divmod(t, NT // 2)
                        nc.sync.dma_start(
                            o2t[:],
                            o2h_o[hh][:, tt * 128:(tt + 1) * 128],
                            transpose=True)
                        st6 = p3st.tile([128, 6], F32, tag="st63")
                        nc.vector.bn_stats(st6[:], o2t[:])
                        nc.vector.bn_aggr(mv[:, 2 * j:2 * j + 2], st6[:])
                        o2ts.append(o2t)
                    std = p3mv.tile([128, GS], F32, tag="std3")
                    nc.scalar.activation(std[:], mv[:, 1:2 * GS:2], AF.Sqrt,
                                         bias=epsb[:])
                    rr = p3mv.tile([128, GS], F32, tag="rr3")
                    nc.vector.reciprocal(rr[:], std[:])
                    nmr = p3mv.tile([128, GS], F32, tag="nmr3")
                    nc.vector.tensor_mul(nmr[:], mv[:, 0:2 * GS:2], rr[:])
                    nc.vector.tensor_scalar_mul(nmr[:], nmr[:], -1.0)
                    for j in range(GS):
                        t = g * GS + j
                        yt = p3y.tile([128, C], BF16, tag="yt")
                        nc.vector.tensor_scalar(
                            yt[:], o2ts[j][:], rr[:, j:j + 1], nmr[:, j:j + 1],
                            ALU.mult, ALU.add)
                        nc.sync.dma_start(y_d[t * 128:(t + 1) * 128, :], yt[:])

            # final matmuls: 4 x 128-pos per psum group
            if upto >= 5:
             with tc.tile_pool(name="p3yT", bufs=3) as p3yT, \
                 tc.tile_pool(name="p3sig", bufs=3) as p3sig, \
                 tc.tile_pool(name="p3out", bufs=3) as p3out, \
                 tc.tile_pool(name="p3ps", bufs=3, space="PSUM") as p3ps:
                for q in range(POS // 512):
                    yT = p3yT.tile([C, 512], BF16, tag="yT")
                    nc.sync.dma_start(yT[:], y_d[q * 512:(q + 1) * 512, :],
                                      transpose=True)
                    xT = p3yT.tile([C, 512], BF16, tag="xT3")
                    nc.sync.dma_start(xT[:], xn1[q * 512:(q + 1) * 512, :],
                                      transpose=True)
                    ps = p3ps.tile([128, 4, 2 * C], F32, tag="ps3")
                    for k in range(4):
                        nc.tensor.matmul(ps[:, k, 0:C],
                                         yT[:, k * 128:(k + 1) * 128],
                                         wfsb[:, 0:C],
                                         start=True, stop=not use_cfin)
                        nc.tensor.matmul(ps[:, k, C:2 * C],
                                         xT[:, k * 128:(k + 1) * 128],
                                         wfsb[:, C:2 * C],
                                         start=True, stop=not use_cfin)
                        if use_cfin:
                            nc.tensor.matmul(ps[:, k, 0:C], ones_sb[:, 0:128],
                                             cfsb[:, 0:C],
                                             start=False, stop=True)
                            nc.tensor.matmul(ps[:, k, C:2 * C],
                                             ones_sb[:, 0:128],
                                             cfsb[:, C:2 * C],
                                             start=False, stop=True)
                    sig = p3sig.tile([128, 4, C], F32, tag="sig3")
                    nc.scalar.activation(sig[:], ps[:, :, C:2 * C], AF.Sigmoid)
                    outt = p3out.tile([128, 4, C], F32, tag="outt")
                    nc.vector.tensor_mul(outt[:], ps[:, :, 0:C], sig[:])
                    dst = out_r.rearrange("(q k p) c -> q k p c", k=4, p=128)
                    nc.sync.dma_start(dst[q].rearrange("k p c -> p k c"),
                                      outt[:])
    nc.compile()
    return nc


def _prep(pair, mask, ln1_w, ln1_b, proj_w, gate_w, ln2_w, ln2_b, out_w,
          gating_w):
    ln1_w = np.asarray(ln1_w, np.float32); ln1_b = np.asarray(ln1_b, np.float32)
    ln2_w = np.asarray(ln2_w, np.float32); ln2_b = np.asarray(ln2_b, np.float32)
    proj_w = np.asarray(proj_w, np.float32); gate_w = np.asarray(gate_w, np.float32)
    out_w = np.asarray(out_w, np.float32); gating_w = np.asarray(gating_w, np.float32)

    # folded weights (ln scale folded in; ln bias becomes additive consts)
    W1 = np.concatenate([proj_w, gate_w], axis=0)          # [512, 128]
    w1t = (W1 * ln1_w[None, :]).T.copy()                   # [128, 512]
    c1 = W1 @ ln1_b                                        # [512]
    wfin = np.concatenate([(out_w * ln2_w[None, :]).T,
                           (gating_w * ln1_w[None, :]).T], axis=1)  # [128, 256]
    cfin = np.concatenate([out_w @ ln2_b, gating_w @ ln1_b])        # [256]

    use_c1 = bool(np.any(c1 != 0.0))
    use_cfin = bool(np.any(cfin != 0.0))
    use_mask = not bool(np.all(mask == 1.0))
    flags = (use_c1, use_cfin, use_mask)
    if flags not in _PROGRAM_CACHE:
        _PROGRAM_CACHE[flags] = _build_program(flags)
    nc = _PROGRAM_CACHE[flags]
    return nc, flags, w1t, c1, wfin, cfin


def kernel(pair, mask, ln1_w, ln1_b, proj_w, gate_w, ln2_w, ln2_b, out_w,
           gating_w):
    pair = np.asarray(pair, dtype=np.float32)
    mask = np.asarray(mask, dtype=np.float32)
    nc, flags, w1t, c1, wfin, cfin = _prep(
        pair, mask, ln1_w, ln1_b, proj_w, gate_w, ln2_w, ln2_b, out_w,
        gating_w)
    use_c1, use_cfin, use_mask = flags

    bf = ml_dtypes.bfloat16
    w1t_b = np.ascontiguousarray(w1t).astype(bf)
    wfin_b = np.ascontiguousarray(wfin).astype(bf)

    in_maps = []
    for c in range(NCORES):
        m = {
            "pair_r": np.ascontiguousarray(
                pair[c * R:(c + 1) * R].reshape(POS, C)),
            "w1t": w1t_b,
            "wfin": wfin_b,
        }
        if use_c1:
            m["c1r"] = c1.reshape(1, -1).astype(bf)
        if use_cfin:
            m["cfr"] = cfin.reshape(1, -1).astype(bf)
        if use_mask:
            m["mask_r"] = np.ascontiguousarray(
                mask[c * R:(c + 1) * R].reshape(POS // 512, 512))
        in_maps.append(m)

    trace = os.environ.get("TRIMUL_TRACE", "") == "1"
    res = run_bass_kernel_spmd(nc, in_maps, core_ids=list(range(NCORES)),
                               trace=trace)
    global LAST_EXEC_NS, LAST_TRACE
    if res.exec_time_ns is not None:
        LAST_EXEC_NS = res.exec_time_ns
    if res.instructions_and_trace is not None:
        LAST_TRACE = res.instructions_and_trace[1]
    out = np.concatenate(
        [res.results[c]["out_r"].reshape(R, N, C) for c in range(NCORES)],
        axis=0)
    return out



# revision 27
# speedup vs baseline: 4344.1967x; 4344.1967x over previous
"""Distributed triangle multiplication (AlphaFold-style) for 8 Trainium2
NeuronCores — v2.

Key structure (SPMD, core r owns pair rows i in [96r, 96r+96)):
  Phase 1: positions processed K-MAJOR (p' = k*96+ii) via strided loads of
    pair; LN1 (bn_stats) -> xn1 [p', c] bf16; xbar-transpose -> xT;
    proj|gate matmuls vs column-permuted folded W1; sigmoid gate; product
    stored fp8 into 4 channel-group A2A buffers [ (s, 8ch), POS ].
  A2A #1 (x4 chunks, fp8): row-sharded -> channel-sharded. Because phase-1
    positions are k-major, the received buffers are directly [k, i] blocks —
    phase 2 needs NO transposes.
  Phase 2 (16 einsum channels/core, 4 per A2A chunk): per channel, stage
    aT/bT k-chunk tiles [128k, 768] with single strided DMAs; outT[j,i]
    = bT.T @ aT accumulated over 6 k-chunks; output written j-major into 2
    half A2A buffers (fired after channels 0-7 and 8-15).
  A2A #2: channel-sharded -> row-sharded, [ (s,8ch), (j,ii) ] bf16.
  Phase 3 (fully channel-major, no transposes except xbar xT reload):
    LN2 stats via PE broadcast-matmuls (ones/128), var/rstd on DVE/ACT,
    yn on gpsimd; final matmuls use yn/xT 96-pos chunks as stationaries
    so psum comes out [pos, co]; sigmoid-gate epilogue; strided f32 store.

Host path: lean PJRT runner (no input concat — views; on-device zero
outputs; output returned as reshaped view of the fetched global array).
"""
import sys

for _p in ("/opt/trn_rl_repo", "/opt/trn_rl_repo/concourse"):
    if _p not in sys.path:
        sys.path.insert(0, _p)

import numpy as np
import ml_dtypes

import concourse.bass as bass
import concourse.tile as tile
from concourse import bacc, mybir

F32 = mybir.dt.float32
BF16 = mybir.dt.bfloat16
FP8 = mybir.dt.float8e4
AF = mybir.ActivationFunctionType
ALU = mybir.AluOpType

N = 768
C = 128
NCORES = 8
R = N // NCORES            # 96 rows per core
POS = R * N                # 73728 positions per core
KC = N // 128              # 6 k-chunks
EPS = 1e-5

_PROGRAM_CACHE = {}
_RUNNER_CACHE = {}
LAST_EXEC_NS = None
LAST_TRACE = None


def _build_program(flags, upto=6, mode="full"):
    use_c1, use_cfin, use_mask = flags
    nc = bacc.Bacc("TRN2", target_bir_lowering=False, debug=False,
                   num_devices=NCORES)

    def _collective(in_ap, out_ap):
        nc.gpsimd.collective_compute("AllToAll", ALU.bypass,
                                     [list(range(NCORES))],
                                     ins=[in_ap], outs=[out_ap])

    p2only = mode == "p2only"
    p3only = mode == "p3only"
    pair_r = nc.dram_tensor("pair_r", [POS, C], F32, kind="ExternalInput").ap()
    w1t = nc.dram_tensor("w1t", [C, 4 * C], BF16, kind="ExternalInput").ap()
    wfin = nc.dram_tensor("wfin", [C, 2 * C], BF16, kind="ExternalInput").ap()
    if use_c1:
        c1r = nc.dram_tensor("c1r", [1, 4 * C], BF16, kind="ExternalInput").ap()
    if use_cfin:
        cfr = nc.dram_tensor("cfr", [1, 2 * C], BF16, kind="ExternalInput").ap()
    if use_mask:
        mask_r = nc.dram_tensor("mask_r", [POS // 512, 512], F32,
                                kind="ExternalInput").ap()

    AB_DT = BF16
    HP = POS // 2
    xn1 = nc.dram_tensor("xn1", [POS, C], BF16,
                         kind="ExternalInput" if (p3only or p2only)
                         else "Internal").ap()
    abi = [nc.dram_tensor(f"abi{g}", [4 * 64, HP], AB_DT).ap()
           for g in range(2)]
    abo = [nc.dram_tensor(f"abo{g}", [4 * 64, HP], AB_DT,
                          kind="ExternalInput" if p2only else "Internal").ap()
           for g in range(2)]
    in2h = [nc.dram_tensor(f"in2h{h}", [64, POS], BF16).ap() for h in range(2)]
    o2h = [nc.dram_tensor(f"o2h{h}", [64, POS], BF16,
                          kind="ExternalInput" if p3only else "Internal").ap()
           for h in range(2)]
    out_r = nc.dram_tensor("out_r", [POS, C], F32, kind="ExternalOutput").ap()
    if p2only:
        upto = 6
    if p3only:
        upto = 6

    with tile.TileContext(nc) as tc:
        with tc.tile_pool(name="consts", bufs=1) as cpool:
            w1sb = cpool.tile([C, 4 * C], BF16)
            nc.sync.dma_start(w1sb[:], w1t[:, :])
            wfsb = cpool.tile([C, 2 * C], BF16)
            nc.sync.dma_start(wfsb[:], wfin[:, :])
            o128 = cpool.tile([C, C], BF16)
            nc.vector.memset(o128[:], 1.0 / C)
            epsb = cpool.tile([128, 1], F32)
            nc.vector.memset(epsb[:], EPS)
            ones_sb = cpool.tile([1, 512], BF16)
            nc.vector.memset(ones_sb[:], 1.0)
            if use_c1:
                c1sb = cpool.tile([1, 4 * C], BF16)
                nc.sync.dma_start(c1sb[:], c1r[:, :])
            if use_cfin:
                cfsb = cpool.tile([1, 2 * C], BF16)
                nc.sync.dma_start(cfsb[:], cfr[:, :])

            # ================= Phase 1: LN1 (k-major) =================
            prv = pair_r.rearrange("(i k) c -> k i c", i=R)
            xn1v = xn1.rearrange("(k i) c -> k i c", i=R)
            GS = 6                 # tiles per stats batch (4 positions each)
            if upto >= 1 and not (p2only or p3only):
             with tc.tile_pool(name="p1x", bufs=GS + 2) as p1x, \
                 tc.tile_pool(name="p1st", bufs=4) as p1st, \
                 tc.tile_pool(name="p1mv", bufs=3) as p1mv, \
                 tc.tile_pool(name="p1xn", bufs=4) as p1xn:
                for kq in range(KC):
                    for gb in range(4):
                        mv = p1mv.tile([128, 48], F32, tag="mv")
                        xts = []
                        for j in range(GS):
                            iig = gb * GS + j
                            xt = p1x.tile([128, 4, C], F32, tag="xt")
                            nc.sync.dma_start(
                                xt[:], prv[kq * 128:(kq + 1) * 128,
                                           iig * 4:(iig + 1) * 4, :])
                            for q in range(4):
                                st6 = p1st.tile([128, 6], F32, tag="st6")
                                nc.vector.bn_stats(st6[:], xt[:, q, :])
                                idx = 4 * j + q
                                nc.vector.bn_aggr(mv[:, 2 * idx:2 * idx + 2],
                                                  st6[:])
                            xts.append(xt)
                        std = p1mv.tile([128, 24], F32, tag="std")
                        nc.scalar.activation(std[:], mv[:, 1:48:2], AF.Sqrt,
                                             bias=epsb[:])
                        rr = p1mv.tile([128, 24], F32, tag="rr")
                        nc.vector.reciprocal(rr[:], std[:])
                        nmr = p1mv.tile([128, 24], F32, tag="nmr")
                        nc.vector.tensor_mul(nmr[:], mv[:, 0:48:2], rr[:])
                        nc.vector.tensor_scalar_mul(nmr[:], nmr[:], -1.0)
                        for j in range(GS):
                            iig = gb * GS + j
                            xnt = p1xn.tile([128, 4, C], BF16, tag="xn")
                            for q in range(4):
                                idx = 4 * j + q
                                nc.gpsimd.tensor_scalar(
                                    xnt[:, q, :], xts[j][:, q, :],
                                    rr[:, idx:idx + 1], nmr[:, idx:idx + 1],
                                    ALU.mult, ALU.add)
                            nc.sync.dma_start(
                                xn1v[kq * 128:(kq + 1) * 128,
                                     iig * 4:(iig + 1) * 4, :], xnt[:])

            # ============ Phase 1: projection + gate ============
            if upto >= 2 and not (p2only or p3only):
             with tc.tile_pool(name="p1xT", bufs=3) as p1xT, \
                 tc.tile_pool(name="p1sig", bufs=3) as p1sig, \
                 tc.tile_pool(name="p1ab", bufs=4) as p1ab, \
                 tc.tile_pool(name="p1ps", bufs=2, space="PSUM") as p1ps:
                for q2 in range(POS // 1024):      # 72
                    xT = p1xT.tile([C, 1024], BF16, tag="xT")
                    nc.sync.dma_start(xT[:], xn1[q2 * 1024:(q2 + 1) * 1024, :],
                                      transpose=True)
                    for h in range(2):
                        ck = 2 * q2 + h
                        ps = p1ps.tile([128, 4, 512], F32, tag="ps")
                        rhs = xT[:, h * 512:(h + 1) * 512]
                        for ob in range(4):
                            nc.tensor.matmul(
                                ps[:, ob, :], w1sb[:, ob * 128:(ob + 1) * 128],
                                rhs, start=True, stop=not use_c1)
                        if use_c1:
                            for ob in range(4):
                                nc.tensor.matmul(
                                    ps[:, ob, :],
                                    c1sb[:, ob * 128:(ob + 1) * 128],
                                    ones_sb[:, 0:512], start=False, stop=True)
                        sig = p1sig.tile([128, 2, 512], F32, tag="sig")
                        nc.scalar.activation(sig[:], ps[:, 2:4, :], AF.Sigmoid)
                        if use_mask:
                            mrow = p1sig.tile([1, 512], F32, tag="mrow")
                            nc.sync.dma_start(mrow[:], mask_r[ck:ck + 1, :])
                            mb = p1sig.tile([128, 512], F32, tag="mb")
                            nc.gpsimd.partition_broadcast(mb[:], mrow[:])
                            nc.vector.tensor_mul(
                                sig[:], sig[:],
                                mb[:].to_broadcast([128, 2, 512]))
                        abt = p1ab.tile([128, 2, 512], AB_DT, tag="abt")
                        nc.vector.tensor_mul(abt[:], ps[:, 0:2, :], sig[:])
                        # abi rows (s, hh, u=uh*8+u'); src partitions (uh,s,u')
                        half, ckl = divmod(ck, POS // 1024)
                        abiv = abi[half].rearrange("(s h u) p -> s h u p",
                                                   s=NCORES, h=2, u=16)
                        for hh in range(2):
                            for uh in range(2):
                                nc.sync.dma_start(
                                    abiv[:, hh, uh * 8:uh * 8 + 8,
                                         ckl * 512:(ckl + 1) * 512],
                                    abt[64 * uh:64 * uh + 64, hh, :])

            # ================= A2A #1 (2 halves) =================
            if upto >= 3 and not (p2only or p3only):
                for g in range(2):
                    _collective(abi[g][:], abo[g][:])

            # ================= Phase 2 =================
            if upto >= 4 and not p3only:
             with tc.tile_pool(name="p2ab", bufs=2 * KC + 2) as p2ab, \
                 tc.tile_pool(name="p2o", bufs=4) as p2o, \
                 tc.tile_pool(name="p2ps", bufs=2, space="PSUM") as p2ps:
                for w in range(16):
                    hh, uh = w // 8, (w % 8) // 4
                    va = hh * 16 + uh * 8 + 2 * (w % 4)
                    # abo[half] rows (r', v=32); cols (k_local*96+ii)
                    abvs = [abo[half].rearrange("(r v) (k i) -> v k r i",
                                                r=NCORES, k=N // 2)
                            for half in range(2)]
                    Ats, Bts = [], []
                    for kq in range(KC):
                        half, kql = divmod(kq, KC // 2)
                        At = p2ab.tile([128, N], AB_DT, tag="At")
                        nc.sync.dma_start(
                            At[:].rearrange("p (r i) -> p r i", r=NCORES),
                            abvs[half][va, kql * 128:(kql + 1) * 128, :, :])
                        Bt = p2ab.tile([128, N], AB_DT, tag="Bt")
                        nc.sync.dma_start(
                            Bt[:].rearrange("p (r i) -> p r i", r=NCORES),
                            abvs[half][va + 1, kql * 128:(kql + 1) * 128, :, :])
                        Ats.append(At)
                        Bts.append(Bt)
                    in2v = in2h[hh].rearrange("(rp wp) (j i) -> rp wp j i",
                                              rp=NCORES, j=N)
                    for jq in range(KC):
                        ps2 = p2ps.tile([128, 2, 512], F32, tag="ps2")
                        for kq in range(KC):
                            lhsT = Bts[kq][:, jq * 128:(jq + 1) * 128]
                            nc.tensor.matmul(ps2[:, 0, 0:384], lhsT,
                                             Ats[kq][:, 0:384],
                                             start=(kq == 0),
                                             stop=(kq == KC - 1))
                            nc.tensor.matmul(ps2[:, 1, 0:384], lhsT,
                                             Ats[kq][:, 384:768],
                                             start=(kq == 0),
                                             stop=(kq == KC - 1))
                        ot = p2o.tile([128, 2, 384], BF16, tag="ot")
                        if (w + jq) % 2 == 0:
                            nc.vector.tensor_copy(ot[:], ps2[:, :, 0:384])
                        else:
                            nc.scalar.activation(ot[:], ps2[:, :, 0:384],
                                                 AF.Copy)
                        for rpp in range(NCORES):
                            nc.sync.dma_start(
                                in2v[rpp, w % 8, jq * 128:(jq + 1) * 128, :],
                                ot[:, rpp // 4, (rpp % 4) * 96:
                                   (rpp % 4 + 1) * 96])
                    if w == 7 and upto >= 5:
                        _collective(in2h[0][:], o2h[0][:])
                if upto >= 5:
                    _collective(in2h[1][:], o2h[1][:])

            # ================= Phase 3 (channel-major) =================
            NT3 = POS // 384       # 192 tiles, each 4 j-columns of 96
            orv = out_r.rearrange("(i j) c -> j i c", i=R)
            if upto >= 6:
             with tc.tile_pool(name="p3x", bufs=4) as p3x, \
                 tc.tile_pool(name="p3s", bufs=6) as p3s, \
                 tc.tile_pool(name="p3y", bufs=4) as p3y, \
                 tc.tile_pool(name="p3o", bufs=4) as p3o, \
                 tc.tile_pool(name="p3st", bufs=2, space="PSUM") as p3st, \
                 tc.tile_pool(name="p3ab", bufs=2, space="PSUM") as p3ab:
                for t in range(NT3):
                    X = p3x.tile([128, 384], BF16, tag="X")
                    nc.sync.dma_start(X[0:64, :],
                                      o2h[0][:, t * 384:(t + 1) * 384])
                    nc.sync.dma_start(X[64:128, :],
                                      o2h[1][:, t * 384:(t + 1) * 384])
                    SQ = p3s.tile([128, 384], BF16, tag="SQ")
                    nc.gpsimd.tensor_mul(SQ[:], X[:], X[:])
                    pst = p3st.tile([128, 2, 512], F32, tag="pst")
                    nc.tensor.matmul(pst[:, 0, 0:384], o128[:], X[:],
                                     start=True, stop=True)
                    nc.tensor.matmul(pst[:, 1, 0:384], o128[:], SQ[:],
                                     start=True, stop=True)
                    msq = p3s.tile([128, 384], F32, tag="msq")
                    nc.scalar.activation(msq[:], pst[:, 0, 0:384], AF.Square)
                    var = p3s.tile([128, 384], F32, tag="var")
                    nc.vector.tensor_sub(var[:], pst[:, 1, 0:384], msq[:])
                    std = p3s.tile([128, 384], F32, tag="std")
                    nc.scalar.activation(std[:], var[:], AF.Sqrt,
                                         bias=epsb[:])
                    rinv = p3s.tile([128, 384], F32, tag="rinv")
                    nc.vector.reciprocal(rinv[:], std[:])
                    xm = p3s.tile([128, 384], F32, tag="xm")
                    nc.vector.tensor_sub(xm[:], X[:], pst[:, 0, 0:384])
                    yn = p3y.tile([128, 384], BF16, tag="yn")
                    nc.gpsimd.tensor_mul(yn[:], xm[:], rinv[:])
                    xTt = p3y.tile([C, 384], BF16, tag="xTt")
                    nc.sync.dma_start(xTt[:], xn1[t * 384:(t + 1) * 384, :],
                                      transpose=True)
                    for jc in range(4):
                        q0 = jc * 96
                        pab = p3ab.tile([96, 2, 512], F32, tag="pab")
                        nc.tensor.matmul(pab[:, 0, 0:C], yn[:, q0:q0 + 96],
                                         wfsb[:, 0:C],
                                         start=True, stop=not use_cfin)
                        nc.tensor.matmul(pab[:, 1, 0:C], xTt[:, q0:q0 + 96],
                                         wfsb[:, C:2 * C],
                                         start=True, stop=not use_cfin)
                        if use_cfin:
                            nc.tensor.matmul(pab[:, 0, 0:C], ones_sb[:, 0:96],
                                             cfsb[:, 0:C],
                                             start=False, stop=True)
                            nc.tensor.matmul(pab[:, 1, 0:C], ones_sb[:, 0:96],
                                             cfsb[:, C:2 * C],
                                             start=False, stop=True)
                        sg = p3o.tile([96, C], F32, tag="sg")
                        nc.scalar.activation(sg[:], pab[:, 1, 0:C], AF.Sigmoid)
                        res = p3o.tile([96, C], F32, tag="res")
                        nc.vector.tensor_mul(res[:], pab[:, 0, 0:C], sg[:])
                        nc.sync.dma_start(orv[t * 4 + jc], res[:])
    nc.compile()
    return nc


def _prep_weights(mask, ln1_w, ln1_b, proj_w, gate_w, ln2_w, ln2_b, out_w,
                  gating_w):
    ln1_w = np.asarray(ln1_w, np.float32); ln1_b = np.asarray(ln1_b, np.float32)
    ln2_w = np.asarray(ln2_w, np.float32); ln2_b = np.asarray(ln2_b, np.float32)
    proj_w = np.asarray(proj_w, np.float32); gate_w = np.asarray(gate_w, np.float32)
    out_w = np.asarray(out_w, np.float32); gating_w = np.asarray(gating_w, np.float32)

    W1 = np.concatenate([proj_w, gate_w], axis=0)          # [512, 128]
    w1t = (W1 * ln1_w[None, :]).T.copy()                   # [128, 512]
    c1 = W1 @ ln1_b                                        # [512]

    # column permutation: permuted col (hh*128 + uh*64 + s*8 + u) holds
    # product channel (32s + 16hh + 8uh + u)
    hhA, uhA, sA, uA = np.meshgrid(np.arange(2), np.arange(2),
                                   np.arange(NCORES), np.arange(8),
                                   indexing="ij")
    perm = (32 * sA + 16 * hhA + 8 * uhA + uA).reshape(-1)
    w1t_p = np.concatenate([w1t[:, perm], w1t[:, 256 + perm]], axis=1)
    c1_p = np.concatenate([c1[perm], c1[256 + perm]])

    # phase-3 channel partition order: p3 = hh2*64 + s*8 + w' <-> 16s+8hh2+w'
    h2A, s2A, w2A = np.meshgrid(np.arange(2), np.arange(NCORES), np.arange(8),
                                indexing="ij")
    p3ch = (16 * s2A + 8 * h2A + w2A).reshape(-1)
    wf_proj = (out_w * ln2_w[None, :]).T                   # [128c, 128co]
    wf_gate = (gating_w * ln1_w[None, :]).T
    wfin = np.concatenate([wf_proj[p3ch, :], wf_gate], axis=1)  # [128, 256]
    cfin = np.concatenate([out_w @ ln2_b, gating_w @ ln1_b])

    use_c1 = bool(np.any(c1_p != 0.0))
    use_cfin = bool(np.any(cfin != 0.0))
    use_mask = not bool(np.all(np.asarray(mask) == 1.0))
    flags = (use_c1, use_cfin, use_mask)
    return flags, w1t_p, c1_p, wfin, cfin


def _get_runner(nc):
    key = id(nc)
    if key in _RUNNER_CACHE:
        return _RUNNER_CACHE[key]

    import jax
    import jax.numpy as jnp
    from jax.sharding import Mesh, PartitionSpec, NamedSharding
    try:
        from jax import shard_map
        def _shard_map(f, mesh, in_specs, out_specs):
            return shard_map(f, mesh=mesh, in_specs=in_specs,
                             out_specs=out_specs, check_vma=False)
    except ImportError:
        from jax.experimental.shard_map import shard_map
        def _shard_map(f, mesh, in_specs, out_specs):
            return shard_map(f, mesh=mesh, in_specs=in_specs,
                             out_specs=out_specs, check_rep=False)
    from concourse.bass2jax import (_bass_exec_p, install_neuronx_cc_hook,
                                    partition_id_tensor)

    install_neuronx_cc_hook()
    partition_name = nc.partition_id_tensor.name if nc.partition_id_tensor else None
    in_names, out_names, out_avals = [], [], []
    for alloc in nc.m.functions[0].allocations:
        if not isinstance(alloc, mybir.MemoryLocationSet):
            continue
        name = alloc.memorylocations[0].name
        if alloc.kind == "ExternalInput":
            if name != partition_name:
                in_names.append(name)
        elif alloc.kind == "ExternalOutput":
            out_names.append(name)
            out_avals.append(jax.core.ShapedArray(
                tuple(alloc.tensor_shape), mybir.dt.np(alloc.dtype)))
    n_params = len(in_names)
    all_in = list(in_names) + list(out_names)
    if partition_name is not None:
        all_in.append(partition_name)

    def _body(*args):
        operands = list(args)
        if partition_name is not None:
            operands.append(partition_id_tensor())
        outs = _bass_exec_p.bind(
            *operands, out_avals=tuple(out_avals), in_names=tuple(all_in),
            out_names=tuple(out_names), lowering_input_output_aliases=(),
            sim_require_finite=True, sim_require_nnan=True, nc=nc)
        return tuple(outs)

    devices = jax.devices()[:NCORES]
    mesh = Mesh(np.asarray(devices), ("core",))
    sh = NamedSharding(mesh, PartitionSpec("core"))
    n_outs = len(out_avals)
    sharded = jax.jit(
        _shard_map(_body, mesh, (PartitionSpec("core"),) * (n_params + n_outs),
                   (PartitionSpec("core"),) * n_outs),
        donate_argnums=tuple(range(n_params, n_params + n_outs)),
        keep_unused=True)
    zero_fns = [
        jax.jit(
            (lambda aval: (lambda: jnp.zeros(
                (NCORES * aval.shape[0],) + tuple(aval.shape[1:]),
                aval.dtype)))(a),
            out_shardings=sh)
        for a in out_avals]
    runner = (sharded, in_names, out_names, sh, zero_fns)
    _RUNNER_CACHE[key] = runner
    return runner


def _host_globals(pair, mask, flags, w1t_p, c1_p, wfin, cfin):
    use_c1, use_cfin, use_mask = flags
    bf = ml_dtypes.bfloat16
    hg = {
        "pair_r": pair.reshape(NCORES * POS, C),
        "w1t": np.concatenate(
            [np.ascontiguousarray(w1t_p).astype(bf)] * NCORES, axis=0),
        "wfin": np.concatenate(
            [np.ascontiguousarray(wfin).astype(bf)] * NCORES, axis=0),
    }
    if use_c1:
        hg["c1r"] = np.concatenate(
            [c1_p.reshape(1, -1).astype(bf)] * NCORES, axis=0)
    if use_cfin:
        hg["cfr"] = np.concatenate(
            [cfin.reshape(1, -1).astype(bf)] * NCORES, axis=0)
    if use_mask:
        # k-major per core: mask[96r+ii, k] -> row-major over (k, ii)
        mk = np.stack([
            np.ascontiguousarray(mask[c * R:(c + 1) * R].T).reshape(
                POS // 512, 512)
            for c in range(NCORES)], axis=0)
        hg["mask_r"] = mk.reshape(NCORES * (POS // 512), 512)
    return hg


def kernel(pair, mask, ln1_w, ln1_b, proj_w, gate_w, ln2_w, ln2_b, out_w,
           gating_w):
    import jax
    global LAST_EXEC_NS
    pair = np.ascontiguousarray(np.asarray(pair, dtype=np.float32))
    mask = np.asarray(mask, dtype=np.float32)
    flags, w1t_p, c1_p, wfin, cfin = _prep_weights(
        mask, ln1_w, ln1_b, proj_w, gate_w, ln2_w, ln2_b, out_w, gating_w)
    if flags not in _PROGRAM_CACHE:
        _PROGRAM_CACHE[flags] = _build_program(flags)
    nc = _PROGRAM_CACHE[flags]
    sharded, in_names, out_names, sh, zero_fns = _get_runner(nc)

    hg = _host_globals(pair, mask, flags, w1t_p, c1_p, wfin, cfin)
    dev_in = [jax.device_put(hg[n], sh) for n in in_names]
    zs = [zf() for zf in zero_fns]
    outs = sharded(*dev_in, *zs)
    out = np.asarray(outs[out_names.index("out_r")])
    return out.reshape(N, N, C)
